# revision 4
# baseline (speedup 1.0000x reference)
"""GCLSTM cell (ChebConv K=1) Trainium2 Bass kernel, 8-core node-parallel.

Math (per node, gates g in [i, f, ct, o]):
    G = x @ W_x[g] + h @ Theta[g] + conv_b[g] + b[g]          # [N, 4*32]
    i = sigmoid(G_i + w_c[0]*c);  f = sigmoid(G_f + w_c[1]*c)
    c_new = f*c + i*tanh(G_ct)
    o = sigmoid(G_o + w_c[2]*c_new);  h_new = o*tanh(c_new)
edge_index / edge_weight are mathematically unused (K=1 ChebConv).

Device layout (per core, 62500 nodes padded to 62976 = 492 subtiles of 128):
  stationary = data (xT / hc1T slices, fp16), moving = small weight mats.
  MM1: hc1_s[65,128].T @ Th[65,128]   -> psum gates  (h@Theta + peephole(c) + bias)
  MMc: hc1_s[65,128].T @ Ic[65,32]    -> psum c copy (node-major c)
  MM2: xT_s[128,128].T @ Wc[128,128]  -> accumulate x@W_x into psum gates
  Pointwise per big-tile of 12 subtiles on ACT (sigmoid/tanh) + DVE (mul/add).
  Outputs stay partition-major in DRAM; host un-interleaves.
"""

import numpy as np

import concourse.bacc as bacc
import concourse.mybir as mybir
import concourse.tile as tile
from concourse.bass import ds
from concourse.bass_utils import run_bass_kernel_spmd

N_NODES = 500000
N_CORES = 8
NPC = N_NODES // N_CORES          # 62500
SUB = 128                         # nodes per subtile (matmul M)
B = 12                            # subtiles per big-tile (PSUM: 3 banks gates + 1 bank c)
NSUB = 492                        # ceil(62500/128)=489, rounded up to mult of B
NPAD = NSUB * SUB                 # 62976
SPAN = 96                         # subtiles per DMA span
NSPAN = (NSUB + SPAN - 1) // SPAN  # 6 (5 full + 1 of 12)

F16 = mybir.dt.float16
F32 = mybir.dt.float32
AF = mybir.ActivationFunctionType
ALU = mybir.AluOpType

_nc_cache = {}


def build_nc():
    if "nc" in _nc_cache:
        return _nc_cache["nc"]
    nc = bacc.Bacc()
    xT = nc.declare_dram_parameter("xT", [128, NPAD], F16, isOutput=False)
    hc1 = nc.declare_dram_parameter("hc1", [65, NPAD], F16, isOutput=False)
    Wc = nc.declare_dram_parameter("Wc", [128, 128], F16, isOutput=False)
    Th = nc.declare_dram_parameter("Th", [65, 128], F16, isOutput=False)
    Ic = nc.declare_dram_parameter("Ic", [65, 32], F16, isOutput=False)
    wc2b = nc.declare_dram_parameter("wc2b", [128, 32], F16, isOutput=False)
    hc_out = nc.declare_dram_parameter(
        "hc_out", [NSPAN, 128, SPAN * 64], F16, isOutput=True)

    with tile.TileContext(nc) as tc:
        with (
            tc.tile_pool(name="const", bufs=1) as cp,
            tc.tile_pool(name="xin", bufs=2) as xp,
            tc.tile_pool(name="hin", bufs=2) as hp,
            tc.tile_pool(name="outp", bufs=2) as op_,
            tc.tile_pool(name="mid", bufs=2) as mp,
            tc.tile_pool(name="pg", bufs=2, space="PSUM") as pgp,
            tc.tile_pool(name="pc", bufs=2, space="PSUM") as pcp,
        ):
            wc_t = cp.tile([128, 128], F16)
            nc.sync.dma_start(wc_t[:], Wc[:])
            th_t = cp.tile([65, 128], F16)
            nc.sync.dma_start(th_t[:], Th[:])
            ic_t = cp.tile([65, 32], F16)
            nc.sync.dma_start(ic_t[:], Ic[:])
            wc2_t = cp.tile([128, 32], F16)
            nc.sync.dma_start(wc2_t[:], wc2b[:])

            for sp in range(NSPAN):
                s0 = sp * SPAN
                nsub = min(SPAN, NSUB - s0)
                nn = nsub * SUB
                x_sp = xp.tile([128, SPAN * SUB], F16, tag="x")
                nc.sync.dma_start(x_sp[:, ds(0, nn)], xT[:, ds(s0 * SUB, nn)])
                h_sp = hp.tile([65, SPAN * SUB], F16, tag="h")
                nc.sync.dma_start(h_sp[:, ds(0, nn)], hc1[:, ds(s0 * SUB, nn)])
                o_sp = op_.tile([128, SPAN, 64], F16, tag="o")

                for bt in range(nsub // B):
                    pg = pgp.tile([128, B, 128], F32, tag="pg")
                    pc = pcp.tile([128, B, 32], F32, tag="pc",
                                  padded_shape=[128, B, 42])
                    for j in range(B):
                        col = (bt * B + j) * SUB
                        hs = h_sp[:, ds(col, SUB)]
                        nc.tensor.matmul(pc[:, j, :], hs, ic_t[:],
                                         start=True, stop=True)
                        nc.tensor.matmul(pg[:, j, :], hs, th_t[:],
                                         start=True, stop=False)
                        nc.tensor.matmul(pg[:, j, :], x_sp[:, ds(col, SUB)],
                                         wc_t[:], start=False, stop=True)

                    # pointwise over the big-tile ([128, B, *] APs)
                    if_t = mp.tile([128, B, 64], F16, tag="if")
                    nc.scalar.activation(if_t[:], pg[:, :, ds(0, 64)], AF.Sigmoid)
                    ct_t = mp.tile([128, B, 32], F16, tag="ct")
                    nc.scalar.activation(ct_t[:], pg[:, :, ds(64, 32)], AF.Tanh)
                    u_t = mp.tile([128, B, 32], F16, tag="u")
                    nc.vector.tensor_tensor(u_t[:], if_t[:, :, ds(32, 32)],
                                            pc[:, :, :], ALU.mult)  # f*c
                    v_t = mp.tile([128, B, 32], F16, tag="v")
                    nc.vector.tensor_tensor(v_t[:], if_t[:, :, ds(0, 32)],
                                            ct_t[:], ALU.mult)      # i*tanh(ct)
                    cn = o_sp[:, ds(bt * B, B), ds(32, 32)]
                    nc.vector.tensor_tensor(cn, u_t[:], v_t[:], ALU.add)
                    t_t = mp.tile([128, B, 32], F16, tag="t")
                    nc.vector.tensor_tensor(
                        t_t[:], cn,
                        wc2_t[:, None, :].to_broadcast([128, B, 32]), ALU.mult)
                    t2_t = mp.tile([128, B, 32], F32, tag="t2")
                    nc.vector.tensor_tensor(t2_t[:], t_t[:],
                                            pg[:, :, ds(96, 32)], ALU.add)
                    og_t = mp.tile([128, B, 32], F16, tag="og")
                    nc.scalar.activation(og_t[:], t2_t[:], AF.Sigmoid)
                    tn_t = mp.tile([128, B, 32], F16, tag="tn")
                    nc.scalar.activation(tn_t[:], cn, AF.Tanh)
                    nc.vector.tensor_tensor(o_sp[:, ds(bt * B, B), ds(0, 32)],
                                            og_t[:], tn_t[:], ALU.mult)

                nc.sync.dma_start(hc_out[sp, :, ds(0, nsub * 64)],
                                  o_sp[:, ds(0, nsub), :])

    nc.finalize()
    _nc_cache["nc"] = nc
    return nc


def _prep_inputs(x, h, c, W_x, Theta, conv_b, w_c, b):
    x = np.asarray(x, dtype=np.float32)
    h = np.asarray(h, dtype=np.float32)
    c = np.asarray(c, dtype=np.float32)
    W_x = np.asarray(W_x, dtype=np.float32)
    Theta = np.asarray(Theta, dtype=np.float32)
    conv_b = np.asarray(conv_b, dtype=np.float32)
    w_c = np.asarray(w_c, dtype=np.float32)
    b = np.asarray(b, dtype=np.float32)

    Wc = np.ascontiguousarray(
        W_x.transpose(1, 0, 2).reshape(128, 128)).astype(np.float16)
    Th = np.zeros((65, 128), np.float32)
    Th[0:32, :] = Theta.transpose(1, 0, 2).reshape(32, 128)
    kk = np.arange(32)
    Th[32 + kk, kk] = w_c[0]
    Th[32 + kk, 32 + kk] = w_c[1]
    Th[64, :] = (conv_b + b).reshape(128)
    Th = Th.astype(np.float16)
    Ic = np.zeros((65, 32), np.float16)
    Ic[32 + kk, kk] = 1.0
    wc2b = np.broadcast_to(w_c[2].astype(np.float16), (128, 32)).copy()

    xf = x.astype(np.float16)
    hf = h.astype(np.float16)
    cf = c.astype(np.float16)
    in_maps = []
    for ci in range(N_CORES):
        sl = slice(ci * NPC, (ci + 1) * NPC)
        xt = np.zeros((128, NPAD), np.float16)
        xt[:, :NPC] = xf[sl].T
        hc1 = np.zeros((65, NPAD), np.float16)
        hc1[0:32, :NPC] = hf[sl].T
        hc1[32:64, :NPC] = cf[sl].T
        hc1[64, :] = 1.0
        in_maps.append({"xT": xt, "hc1": hc1, "Wc": Wc, "Th": Th,
                       "Ic": Ic, "wc2b": wc2b})
    return in_maps


def _decode(results):
    hs, cs = [], []
    for ci in range(N_CORES):
        out = np.asarray(results[ci]["hc_out"])  # [NSPAN, 128, SPAN*64] f16
        arr = out.reshape(NSPAN, 128, SPAN, 64).transpose(0, 2, 1, 3)
        arr = arr.reshape(NSPAN * SPAN * 128, 64)[:NPC]
        hs.append(arr[:, 0:32].astype(np.float32))
        cs.append(arr[:, 32:64].astype(np.float32))
    return np.concatenate(hs, axis=0), np.concatenate(cs, axis=0)


def _run(inputs, trace=False):
    nc = build_nc()
    in_maps = _prep_inputs(
        inputs["x"], inputs["h"], inputs["c"], inputs["W_x"],
        inputs["Theta"], inputs["conv_b"], inputs["w_c"], inputs["b"])
    res = run_bass_kernel_spmd(nc, in_maps, list(range(N_CORES)), trace=trace)
    h_new, c_new = _decode(res.results)
    return (h_new, c_new), res


def kernel(**inputs):
    (h_new, c_new), _ = _run(inputs, trace=False)
    return (h_new, c_new)


def kernel_profiled(**inputs):
    (h_new, c_new), res = _run(inputs, trace=True)
    return (h_new, c_new), res


# revision 8
# speedup vs baseline: 1.0853x; 1.0853x over previous
"""GCLSTM cell (ChebConv K=1) Trainium2 Bass kernel, 8-core node-parallel.

Math (per node, gates g in [i, f, ct, o]):
    G = x @ W_x[g] + h @ Theta[g] + conv_b[g] + b[g]          # [N, 4*32]
    i = sigmoid(G_i + w_c[0]*c);  f = sigmoid(G_f + w_c[1]*c)
    c_new = f*c + i*tanh(G_ct)
    o = sigmoid(G_o + w_c[2]*c_new);  h_new = o*tanh(c_new)
edge_index / edge_weight are mathematically unused (K=1 ChebConv).

Device layout (per core, 62500 nodes padded to 62976 = 492 subtiles of 128):
  stationary = data (xT / hc1T slices, fp16), moving = small weight mats.
  MM1: hc1_s[65,128].T @ Th[65,128]   -> psum gates  (h@Theta + peephole(c) + bias)
  MMc: hc1_s[65,128].T @ Ic[65,32]    -> psum c copy (node-major c)
  MM2: xT_s[128,128].T @ Wc[128,128]  -> accumulate x@W_x into psum gates
  Pointwise per big-tile of 12 subtiles on ACT (sigmoid/tanh) + DVE (mul/add).
  Outputs stay partition-major in DRAM; host un-interleaves.
"""

import numpy as np

import concourse.bacc as bacc
import concourse.mybir as mybir
import concourse.tile as tile
from concourse.bass import ds
from concourse.bass_utils import run_bass_kernel_spmd

N_NODES = 500000
N_CORES = 8
NPC = N_NODES // N_CORES          # 62500
SUB = 128                         # nodes per subtile (matmul M)
B = 12                            # subtiles per big-tile (PSUM: 3 banks gates + 1 bank c)
NSUB = 492                        # ceil(62500/128)=489, rounded up to mult of B
NPAD = NSUB * SUB                 # 62976
CHUNK = 24                        # subtiles per input-DMA chunk (2 big-tiles)
NCHUNK = (NSUB + CHUNK - 1) // CHUNK  # 21 (20 full + 1 of 12)

F16 = mybir.dt.float16
F32 = mybir.dt.float32
AF = mybir.ActivationFunctionType
ALU = mybir.AluOpType

_nc_cache = {}


def build_nc():
    if "nc" in _nc_cache:
        return _nc_cache["nc"]
    nc = bacc.Bacc()
    xT = nc.declare_dram_parameter("xT", [128, NPAD], F16, isOutput=False)
    hc1 = nc.declare_dram_parameter("hc1", [65, NPAD], F16, isOutput=False)
    Wc = nc.declare_dram_parameter("Wc", [128, 128], F16, isOutput=False)
    Th = nc.declare_dram_parameter("Th", [65, 128], F16, isOutput=False)
    Ic = nc.declare_dram_parameter("Ic", [65, 32], F16, isOutput=False)
    wc2b = nc.declare_dram_parameter("wc2b", [128, 32], F16, isOutput=False)
    hc_out = nc.declare_dram_parameter(
        "hc_out", [NCHUNK, 128, CHUNK * 64], F16, isOutput=True)

    with tile.TileContext(nc) as tc:
        with (
            tc.tile_pool(name="const", bufs=1) as cp,
            tc.tile_pool(name="xin", bufs=4) as xp,
            tc.tile_pool(name="hin", bufs=4) as hp,
            tc.tile_pool(name="outp", bufs=3) as op_,
            tc.tile_pool(name="mid", bufs=2) as mp,
            tc.tile_pool(name="pg", bufs=2, space="PSUM") as pgp,
            tc.tile_pool(name="pc", bufs=2, space="PSUM") as pcp,
        ):
            wc_t = cp.tile([128, 128], F16)
            nc.scalar.dma_start(wc_t[:], Wc[:])
            th_t = cp.tile([65, 128], F16)
            nc.scalar.dma_start(th_t[:], Th[:])
            ic_t = cp.tile([65, 32], F16)
            nc.scalar.dma_start(ic_t[:], Ic[:])
            wc2_t = cp.tile([128, 32], F16)
            nc.scalar.dma_start(wc2_t[:], wc2b[:])

            for c in range(NCHUNK):
                s0 = c * CHUNK
                nsub = min(CHUNK, NSUB - s0)
                nn = nsub * SUB
                x_ch = xp.tile([128, CHUNK * SUB], F16, tag="x")
                nc.sync.dma_start(x_ch[:, ds(0, nn)], xT[:, ds(s0 * SUB, nn)])
                h_ch = hp.tile([65, CHUNK * SUB], F16, tag="h")
                nc.sync.dma_start(h_ch[:, ds(0, nn)], hc1[:, ds(s0 * SUB, nn)])
                o_ch = op_.tile([128, CHUNK, 64], F16, tag="o")

                for bt in range(nsub // B):
                    pg = pgp.tile([128, B, 128], F32, tag="pg")
                    pc = pcp.tile([128, B, 32], F32, tag="pc",
                                  padded_shape=[128, B, 42])
                    for j in range(B):
                        col = (bt * B + j) * SUB
                        hs = h_ch[:, ds(col, SUB)]
                        nc.tensor.matmul(pc[:, j, :], hs, ic_t[:],
                                         start=True, stop=True)
                        nc.tensor.matmul(pg[:, j, :], hs, th_t[:],
                                         start=True, stop=False)
                        nc.tensor.matmul(pg[:, j, :], x_ch[:, ds(col, SUB)],
                                         wc_t[:], start=False, stop=True)

                    # pointwise over the big-tile ([128, B, *] APs)
                    if_t = mp.tile([128, B, 64], F16, tag="if")
                    nc.scalar.activation(if_t[:], pg[:, :, ds(0, 64)], AF.Sigmoid)
                    ct_t = mp.tile([128, B, 32], F16, tag="ct")
                    nc.scalar.activation(ct_t[:], pg[:, :, ds(64, 32)], AF.Tanh)
                    u_t = mp.tile([128, B, 32], F16, tag="u")
                    nc.vector.tensor_tensor(u_t[:], if_t[:, :, ds(32, 32)],
                                            pc[:, :, :], ALU.mult)  # f*c
                    v_t = mp.tile([128, B, 32], F16, tag="v")
                    nc.vector.tensor_tensor(v_t[:], if_t[:, :, ds(0, 32)],
                                            ct_t[:], ALU.mult)      # i*tanh(ct)
                    cn = o_ch[:, ds(bt * B, B), ds(32, 32)]
                    nc.vector.tensor_tensor(cn, u_t[:], v_t[:], ALU.add)
                    t_t = mp.tile([128, B, 32], F16, tag="t")
                    nc.vector.tensor_tensor(
                        t_t[:], cn,
                        wc2_t[:, None, :].to_broadcast([128, B, 32]), ALU.mult)
                    t2_t = mp.tile([128, B, 32], F32, tag="t2")
                    nc.vector.tensor_tensor(t2_t[:], t_t[:],
                                            pg[:, :, ds(96, 32)], ALU.add)
                    og_t = mp.tile([128, B, 32], F16, tag="og")
                    nc.scalar.activation(og_t[:], t2_t[:], AF.Sigmoid)
                    tn_t = mp.tile([128, B, 32], F16, tag="tn")
                    nc.scalar.activation(tn_t[:], cn, AF.Tanh)
                    nc.vector.tensor_tensor(o_ch[:, ds(bt * B, B), ds(0, 32)],
                                            og_t[:], tn_t[:], ALU.mult)

                nc.gpsimd.dma_start(hc_out[c, :, ds(0, nsub * 64)],
                                    o_ch[:, ds(0, nsub), :])

    nc.finalize()
    _nc_cache["nc"] = nc
    return nc


def _prep_inputs(x, h, c, W_x, Theta, conv_b, w_c, b):
    x = np.asarray(x, dtype=np.float32)
    h = np.asarray(h, dtype=np.float32)
    c = np.asarray(c, dtype=np.float32)
    W_x = np.asarray(W_x, dtype=np.float32)
    Theta = np.asarray(Theta, dtype=np.float32)
    conv_b = np.asarray(conv_b, dtype=np.float32)
    w_c = np.asarray(w_c, dtype=np.float32)
    b = np.asarray(b, dtype=np.float32)

    Wc = np.ascontiguousarray(
        W_x.transpose(1, 0, 2).reshape(128, 128)).astype(np.float16)
    Th = np.zeros((65, 128), np.float32)
    Th[0:32, :] = Theta.transpose(1, 0, 2).reshape(32, 128)
    kk = np.arange(32)
    Th[32 + kk, kk] = w_c[0]
    Th[32 + kk, 32 + kk] = w_c[1]
    Th[64, :] = (conv_b + b).reshape(128)
    Th = Th.astype(np.float16)
    Ic = np.zeros((65, 32), np.float16)
    Ic[32 + kk, kk] = 1.0
    wc2b = np.broadcast_to(w_c[2].astype(np.float16), (128, 32)).copy()

    xf = x.astype(np.float16)
    hf = h.astype(np.float16)
    cf = c.astype(np.float16)
    in_maps = []
    for ci in range(N_CORES):
        sl = slice(ci * NPC, (ci + 1) * NPC)
        xt = np.zeros((128, NPAD), np.float16)
        xt[:, :NPC] = xf[sl].T
        hc1 = np.zeros((65, NPAD), np.float16)
        hc1[0:32, :NPC] = hf[sl].T
        hc1[32:64, :NPC] = cf[sl].T
        hc1[64, :] = 1.0
        in_maps.append({"xT": xt, "hc1": hc1, "Wc": Wc, "Th": Th,
                       "Ic": Ic, "wc2b": wc2b})
    return in_maps


def _decode(results):
    hs, cs = [], []
    for ci in range(N_CORES):
        out = np.asarray(results[ci]["hc_out"])  # [NCHUNK, 128, CHUNK*64] f16
        arr = out.reshape(NCHUNK, 128, CHUNK, 64).transpose(0, 2, 1, 3)
        arr = arr.reshape(NCHUNK * CHUNK * 128, 64)[:NPC]
        hs.append(arr[:, 0:32].astype(np.float32))
        cs.append(arr[:, 32:64].astype(np.float32))
    return np.concatenate(hs, axis=0), np.concatenate(cs, axis=0)


def _run(inputs, trace=False):
    nc = build_nc()
    in_maps = _prep_inputs(
        inputs["x"], inputs["h"], inputs["c"], inputs["W_x"],
        inputs["Theta"], inputs["conv_b"], inputs["w_c"], inputs["b"])
    res = run_bass_kernel_spmd(nc, in_maps, list(range(N_CORES)), trace=trace)
    h_new, c_new = _decode(res.results)
    return (h_new, c_new), res


def kernel(**inputs):
    (h_new, c_new), _ = _run(inputs, trace=False)
    return (h_new, c_new)


def kernel_profiled(**inputs):
    (h_new, c_new), res = _run(inputs, trace=True)
    return (h_new, c_new), res


# revision 9
# speedup vs baseline: 1.1917x; 1.0981x over previous
"""GCLSTM cell (ChebConv K=1) Trainium2 Bass kernel, 8-core node-parallel.

Math (per node, gates g in [i, f, ct, o]):
    G = x @ W_x[g] + h @ Theta[g] + conv_b[g] + b[g]          # [N, 4*32]
    i = sigmoid(G_i + w_c[0]*c);  f = sigmoid(G_f + w_c[1]*c)
    c_new = f*c + i*tanh(G_ct)
    o = sigmoid(G_o + w_c[2]*c_new);  h_new = o*tanh(c_new)
edge_index / edge_weight are mathematically unused (K=1 ChebConv).

Device layout (per core, 62500 nodes padded to 63488 = 496 subtiles of 128):
  stationary = data (xT / hc1 slices, fp16), moving = small weight mats.
  MM1: hc1_s[65,128].T @ Th[65,128]   -> psum gates  (h@Theta + peephole(c) + bias)
  MM2: xT_s[128,128].T @ Wc[128,128]  -> accumulate x@W_x into psum gates
  Node-major c arrives pre-transposed from host (cnm) - no PE transpose.
  Pointwise per big-tile of 16 subtiles on ACT (sigmoid/tanh) + DVE (mul/add).
  Outputs stay partition-major in DRAM; host un-interleaves.
"""

import numpy as np

import concourse.bacc as bacc
import concourse.mybir as mybir
import concourse.tile as tile
from concourse.bass import ds
from concourse.bass_utils import run_bass_kernel_spmd

N_NODES = 500000
N_CORES = 8
NPC = N_NODES // N_CORES          # 62500
SUB = 128                         # nodes per subtile (matmul M)
B = 16                            # subtiles per big-tile (pg = 4 PSUM banks)
NSUB = 496                        # ceil(62500/128)=489, rounded up to mult of B
NPAD = NSUB * SUB                 # 63488
CHUNK = 32                        # subtiles per input-DMA chunk (2 big-tiles)
SIZES = [16] + [32] * 15          # per-chunk subtile counts (small first chunk)
NCHUNK = len(SIZES)               # 16

F16 = mybir.dt.float16
F32 = mybir.dt.float32
AF = mybir.ActivationFunctionType
ALU = mybir.AluOpType

_nc_cache = {}


def build_nc():
    if "nc" in _nc_cache:
        return _nc_cache["nc"]
    nc = bacc.Bacc()
    xT = nc.declare_dram_parameter("xT", [128, NPAD], F16, isOutput=False)
    hc1 = nc.declare_dram_parameter("hc1", [65, NPAD], F16, isOutput=False)
    cnm = nc.declare_dram_parameter(
        "cnm", [NCHUNK, 128, CHUNK * 32], F16, isOutput=False)
    Wc = nc.declare_dram_parameter("Wc", [128, 128], F16, isOutput=False)
    Th = nc.declare_dram_parameter("Th", [65, 128], F16, isOutput=False)
    wc2b = nc.declare_dram_parameter("wc2b", [128, 32], F16, isOutput=False)
    hc_out = nc.declare_dram_parameter(
        "hc_out", [NCHUNK, 128, CHUNK * 64], F16, isOutput=True)

    with tile.TileContext(nc) as tc:
        with (
            tc.tile_pool(name="const", bufs=1) as cp,
            tc.tile_pool(name="xin", bufs=6) as xp,
            tc.tile_pool(name="hin", bufs=6) as hp,
            tc.tile_pool(name="cin", bufs=6) as cip,
            tc.tile_pool(name="outp", bufs=3) as op_,
            tc.tile_pool(name="mid", bufs=2) as mp,
            tc.tile_pool(name="pg", bufs=2, space="PSUM") as pgp,
        ):
            wc_t = cp.tile([128, 128], F16)
            nc.scalar.dma_start(wc_t[:], Wc[:])
            th_t = cp.tile([65, 128], F16)
            nc.scalar.dma_start(th_t[:], Th[:])
            wc2_t = cp.tile([128, 32], F16)
            nc.scalar.dma_start(wc2_t[:], wc2b[:])

            s0 = 0
            for c, nsub in enumerate(SIZES):
                nn = nsub * SUB
                h_ch = hp.tile([65, CHUNK * SUB], F16, tag="h")
                nc.sync.dma_start(h_ch[:, ds(0, nn)], hc1[:, ds(s0 * SUB, nn)])
                x_ch = xp.tile([128, CHUNK * SUB], F16, tag="x")
                nc.sync.dma_start(x_ch[:, ds(0, nn)], xT[:, ds(s0 * SUB, nn)])
                c_ch = cip.tile([128, CHUNK, 32], F16, tag="c")
                nc.sync.dma_start(c_ch[:, ds(0, nsub), :],
                                  cnm[c, :, ds(0, nsub * 32)])
                o_ch = op_.tile([128, CHUNK, 64], F16, tag="o")

                for bt in range(nsub // B):
                    pg = pgp.tile([128, B, 128], F32, tag="pg")
                    for j in range(B):
                        col = (bt * B + j) * SUB
                        nc.tensor.matmul(pg[:, j, :], h_ch[:, ds(col, SUB)],
                                         th_t[:], start=True, stop=False)
                        nc.tensor.matmul(pg[:, j, :], x_ch[:, ds(col, SUB)],
                                         wc_t[:], start=False, stop=True)

                    # pointwise over the big-tile ([128, B, *] APs)
                    if_t = mp.tile([128, B, 64], F16, tag="if")
                    nc.scalar.activation(if_t[:], pg[:, :, ds(0, 64)], AF.Sigmoid)
                    ct_t = mp.tile([128, B, 32], F16, tag="ct")
                    nc.scalar.activation(ct_t[:], pg[:, :, ds(64, 32)], AF.Tanh)
                    u_t = mp.tile([128, B, 32], F16, tag="u")
                    nc.vector.tensor_tensor(u_t[:], if_t[:, :, ds(32, 32)],
                                            c_ch[:, ds(bt * B, B), :],
                                            ALU.mult)               # f*c
                    v_t = mp.tile([128, B, 32], F16, tag="v")
                    nc.vector.tensor_tensor(v_t[:], if_t[:, :, ds(0, 32)],
                                            ct_t[:], ALU.mult)      # i*tanh(ct)
                    cn = o_ch[:, ds(bt * B, B), ds(32, 32)]
                    nc.vector.tensor_tensor(cn, u_t[:], v_t[:], ALU.add)
                    t_t = mp.tile([128, B, 32], F16, tag="t")
                    nc.vector.tensor_tensor(
                        t_t[:], cn,
                        wc2_t[:, None, :].to_broadcast([128, B, 32]), ALU.mult)
                    t2_t = mp.tile([128, B, 32], F32, tag="t2")
                    nc.vector.tensor_tensor(t2_t[:], t_t[:],
                                            pg[:, :, ds(96, 32)], ALU.add)
                    og_t = mp.tile([128, B, 32], F16, tag="og")
                    nc.scalar.activation(og_t[:], t2_t[:], AF.Sigmoid)
                    tn_t = mp.tile([128, B, 32], F16, tag="tn")
                    nc.scalar.activation(tn_t[:], cn, AF.Tanh)
                    nc.vector.tensor_tensor(o_ch[:, ds(bt * B, B), ds(0, 32)],
                                            og_t[:], tn_t[:], ALU.mult)

                nc.gpsimd.dma_start(hc_out[c, :, ds(0, nsub * 64)],
                                    o_ch[:, ds(0, nsub), :])
                s0 += nsub

    nc.finalize()
    _nc_cache["nc"] = nc
    return nc


def _prep_inputs(x, h, c, W_x, Theta, conv_b, w_c, b):
    x = np.asarray(x, dtype=np.float32)
    h = np.asarray(h, dtype=np.float32)
    c = np.asarray(c, dtype=np.float32)
    W_x = np.asarray(W_x, dtype=np.float32)
    Theta = np.asarray(Theta, dtype=np.float32)
    conv_b = np.asarray(conv_b, dtype=np.float32)
    w_c = np.asarray(w_c, dtype=np.float32)
    b = np.asarray(b, dtype=np.float32)

    Wc = np.ascontiguousarray(
        W_x.transpose(1, 0, 2).reshape(128, 128)).astype(np.float16)
    Th = np.zeros((65, 128), np.float32)
    Th[0:32, :] = Theta.transpose(1, 0, 2).reshape(32, 128)
    kk = np.arange(32)
    Th[32 + kk, kk] = w_c[0]
    Th[32 + kk, 32 + kk] = w_c[1]
    Th[64, :] = (conv_b + b).reshape(128)
    Th = Th.astype(np.float16)
    wc2b = np.broadcast_to(w_c[2].astype(np.float16), (128, 32)).copy()

    xf = x.astype(np.float16)
    hf = h.astype(np.float16)
    cf = c.astype(np.float16)
    in_maps = []
    for ci in range(N_CORES):
        sl = slice(ci * NPC, (ci + 1) * NPC)
        xt = np.zeros((128, NPAD), np.float16)
        xt[:, :NPC] = xf[sl].T
        hc1 = np.zeros((65, NPAD), np.float16)
        hc1[0:32, :NPC] = hf[sl].T
        hc1[32:64, :NPC] = cf[sl].T
        hc1[64, :] = 1.0
        cpad = np.zeros((NPAD, 32), np.float16)
        cpad[:NPC] = cf[sl]
        t = cpad.reshape(NSUB, 128, 32).transpose(1, 0, 2)  # [128, NSUB, 32]
        cnm = np.zeros((NCHUNK, 128, CHUNK * 32), np.float16)
        s0 = 0
        for cc, nsub in enumerate(SIZES):
            cnm[cc, :, :nsub * 32] = t[:, s0:s0 + nsub].reshape(128, nsub * 32)
            s0 += nsub
        in_maps.append({"xT": xt, "hc1": hc1, "cnm": cnm, "Wc": Wc, "Th": Th,
                        "wc2b": wc2b})
    return in_maps


def _decode(results):
    hs, cs = [], []
    for ci in range(N_CORES):
        out = np.asarray(results[ci]["hc_out"])  # [NCHUNK, 128, CHUNK*64] f16
        parts = []
        for cc, nsub in enumerate(SIZES):
            a = out[cc, :, :nsub * 64].reshape(128, nsub, 64)
            parts.append(a.transpose(1, 0, 2).reshape(nsub * 128, 64))
        arr = np.concatenate(parts, axis=0)[:NPC]
        hs.append(arr[:, 0:32].astype(np.float32))
        cs.append(arr[:, 32:64].astype(np.float32))
    return np.concatenate(hs, axis=0), np.concatenate(cs, axis=0)


def _run(inputs, trace=False):
    nc = build_nc()
    in_maps = _prep_inputs(
        inputs["x"], inputs["h"], inputs["c"], inputs["W_x"],
        inputs["Theta"], inputs["conv_b"], inputs["w_c"], inputs["b"])
    res = run_bass_kernel_spmd(nc, in_maps, list(range(N_CORES)), trace=trace)
    h_new, c_new = _decode(res.results)
    return (h_new, c_new), res


def kernel(**inputs):
    (h_new, c_new), _ = _run(inputs, trace=False)
    return (h_new, c_new)


def kernel_profiled(**inputs):
    (h_new, c_new), res = _run(inputs, trace=True)
    return (h_new, c_new), res


# revision 11
# speedup vs baseline: 1.1966x; 1.0041x over previous
"""GCLSTM cell (ChebConv K=1) Trainium2 Bass kernel, 8-core node-parallel.

Math (per node, gates g in [i, f, ct, o]):
    G = x @ W_x[g] + h @ Theta[g] + conv_b[g] + b[g]          # [N, 4*32]
    i = sigmoid(G_i + w_c[0]*c);  f = sigmoid(G_f + w_c[1]*c)
    c_new = f*c + i*tanh(G_ct)
    o = sigmoid(G_o + w_c[2]*c_new);  h_new = o*tanh(c_new)
edge_index / edge_weight are mathematically unused (K=1 ChebConv).

Device layout (per core, 62500 nodes padded to 63488 = 496 subtiles of 128):
  stationary = data (xT / hc1 slices, fp16), moving = small weight mats.
  MM1: hc1_s[65,128].T @ Th[65,128]   -> psum gates  (h@Theta + peephole(c) + bias)
  MM2: xT_s[128,128].T @ Wc[128,128]  -> accumulate x@W_x into psum gates
  Node-major c arrives pre-transposed from host (cnm) - no PE transpose.
  Pointwise per big-tile of 16 subtiles on ACT (sigmoid/tanh) + DVE (mul/add).
  Outputs stay partition-major in DRAM; host un-interleaves.
"""

import numpy as np

import concourse.bacc as bacc
import concourse.mybir as mybir
import concourse.tile as tile
from concourse.bass import ds
from concourse.bass_utils import run_bass_kernel_spmd

N_NODES = 500000
N_CORES = 8
NPC = N_NODES // N_CORES          # 62500
SUB = 128                         # nodes per subtile (matmul M)
B = 16                            # subtiles per big-tile (pg = 4 PSUM banks)
NSUB = 496                        # ceil(62500/128)=489, rounded up to mult of B
NPAD = NSUB * SUB                 # 63488
CHUNK = 32                        # subtiles per input-DMA chunk (2 big-tiles)
SIZES = [16] + [32] * 14 + [16, 16]   # small first/last chunks (startup/tail)
NCHUNK = len(SIZES)               # 17

F16 = mybir.dt.float16
F32 = mybir.dt.float32
AF = mybir.ActivationFunctionType
ALU = mybir.AluOpType

_nc_cache = {}


def build_nc():
    if "nc" in _nc_cache:
        return _nc_cache["nc"]
    nc = bacc.Bacc()
    xT = nc.declare_dram_parameter("xT", [128, NPAD], F16, isOutput=False)
    hc1 = nc.declare_dram_parameter("hc1", [65, NPAD], F16, isOutput=False)
    cnm = nc.declare_dram_parameter(
        "cnm", [NCHUNK, 128, CHUNK * 32], F16, isOutput=False)
    Wc = nc.declare_dram_parameter("Wc", [128, 128], F16, isOutput=False)
    Th = nc.declare_dram_parameter("Th", [65, 128], F16, isOutput=False)
    wc2b = nc.declare_dram_parameter("wc2b", [128, 32], F16, isOutput=False)
    hc_out = nc.declare_dram_parameter(
        "hc_out", [NCHUNK, 128, CHUNK * 64], F16, isOutput=True)

    with tile.TileContext(nc) as tc:
        with (
            tc.tile_pool(name="const", bufs=1) as cp,
            tc.tile_pool(name="xin", bufs=6) as xp,
            tc.tile_pool(name="hin", bufs=6) as hp,
            tc.tile_pool(name="cin", bufs=6) as cip,
            tc.tile_pool(name="outp", bufs=3) as op_,
            tc.tile_pool(name="mid", bufs=2) as mp,
            tc.tile_pool(name="pg", bufs=2, space="PSUM") as pgp,
        ):
            wc_t = cp.tile([128, 128], F16)
            nc.scalar.dma_start(wc_t[:], Wc[:])
            th_t = cp.tile([65, 128], F16)
            nc.scalar.dma_start(th_t[:], Th[:])
            wc2_t = cp.tile([128, 32], F16)
            nc.scalar.dma_start(wc2_t[:], wc2b[:])

            s0 = 0
            for c, nsub in enumerate(SIZES):
                nn = nsub * SUB
                h_ch = hp.tile([65, CHUNK * SUB], F16, tag="h")
                nc.sync.dma_start(h_ch[:, ds(0, nn)], hc1[:, ds(s0 * SUB, nn)])
                x_ch = xp.tile([128, CHUNK * SUB], F16, tag="x")
                nc.sync.dma_start(x_ch[:, ds(0, nn)], xT[:, ds(s0 * SUB, nn)])
                c_ch = cip.tile([128, CHUNK, 32], F16, tag="c")
                nc.sync.dma_start(c_ch[:, ds(0, nsub), :],
                                  cnm[c, :, ds(0, nsub * 32)])
                o_ch = op_.tile([128, CHUNK, 64], F16, tag="o")

                for bt in range(nsub // B):
                    pg = pgp.tile([128, B, 128], F32, tag="pg")
                    for j in range(B):
                        col = (bt * B + j) * SUB
                        nc.tensor.matmul(pg[:, j, :], h_ch[:, ds(col, SUB)],
                                         th_t[:], start=True, stop=False)
                        nc.tensor.matmul(pg[:, j, :], x_ch[:, ds(col, SUB)],
                                         wc_t[:], start=False, stop=True)

                    # pointwise over the big-tile ([128, B, *] APs)
                    if_t = mp.tile([128, B, 64], F16, tag="if")
                    nc.scalar.activation(if_t[:], pg[:, :, ds(0, 64)], AF.Sigmoid)
                    ct_t = mp.tile([128, B, 32], F16, tag="ct")
                    nc.scalar.activation(ct_t[:], pg[:, :, ds(64, 32)], AF.Tanh)
                    u_t = mp.tile([128, B, 32], F16, tag="u")
                    nc.vector.tensor_tensor(u_t[:], if_t[:, :, ds(32, 32)],
                                            c_ch[:, ds(bt * B, B), :],
                                            ALU.mult)               # f*c
                    v_t = mp.tile([128, B, 32], F16, tag="v")
                    nc.vector.tensor_tensor(v_t[:], if_t[:, :, ds(0, 32)],
                                            ct_t[:], ALU.mult)      # i*tanh(ct)
                    cn = o_ch[:, ds(bt * B, B), ds(32, 32)]
                    nc.vector.tensor_tensor(cn, u_t[:], v_t[:], ALU.add)
                    t_t = mp.tile([128, B, 32], F16, tag="t")
                    nc.vector.tensor_tensor(
                        t_t[:], cn,
                        wc2_t[:, None, :].to_broadcast([128, B, 32]), ALU.mult)
                    t2_t = mp.tile([128, B, 32], F32, tag="t2")
                    nc.vector.tensor_tensor(t2_t[:], t_t[:],
                                            pg[:, :, ds(96, 32)], ALU.add)
                    og_t = mp.tile([128, B, 32], F16, tag="og")
                    nc.scalar.activation(og_t[:], t2_t[:], AF.Sigmoid)
                    tn_t = mp.tile([128, B, 32], F16, tag="tn")
                    nc.scalar.activation(tn_t[:], cn, AF.Tanh)
                    nc.vector.tensor_tensor(o_ch[:, ds(bt * B, B), ds(0, 32)],
                                            og_t[:], tn_t[:], ALU.mult)
                    nc.gpsimd.dma_start(hc_out[c, :, ds(bt * B * 64, B * 64)],
                                        o_ch[:, ds(bt * B, B), :])
                s0 += nsub

    nc.finalize()
    _nc_cache["nc"] = nc
    return nc


def _prep_inputs(x, h, c, W_x, Theta, conv_b, w_c, b):
    x = np.asarray(x, dtype=np.float32)
    h = np.asarray(h, dtype=np.float32)
    c = np.asarray(c, dtype=np.float32)
    W_x = np.asarray(W_x, dtype=np.float32)
    Theta = np.asarray(Theta, dtype=np.float32)
    conv_b = np.asarray(conv_b, dtype=np.float32)
    w_c = np.asarray(w_c, dtype=np.float32)
    b = np.asarray(b, dtype=np.float32)

    Wc = np.ascontiguousarray(
        W_x.transpose(1, 0, 2).reshape(128, 128)).astype(np.float16)
    Th = np.zeros((65, 128), np.float32)
    Th[0:32, :] = Theta.transpose(1, 0, 2).reshape(32, 128)
    kk = np.arange(32)
    Th[32 + kk, kk] = w_c[0]
    Th[32 + kk, 32 + kk] = w_c[1]
    Th[64, :] = (conv_b + b).reshape(128)
    Th = Th.astype(np.float16)
    wc2b = np.broadcast_to(w_c[2].astype(np.float16), (128, 32)).copy()

    xf = x.astype(np.float16)
    hf = h.astype(np.float16)
    cf = c.astype(np.float16)
    in_maps = []
    for ci in range(N_CORES):
        sl = slice(ci * NPC, (ci + 1) * NPC)
        xt = np.zeros((128, NPAD), np.float16)
        xt[:, :NPC] = xf[sl].T
        hc1 = np.zeros((65, NPAD), np.float16)
        hc1[0:32, :NPC] = hf[sl].T
        hc1[32:64, :NPC] = cf[sl].T
        hc1[64, :] = 1.0
        cpad = np.zeros((NPAD, 32), np.float16)
        cpad[:NPC] = cf[sl]
        t = cpad.reshape(NSUB, 128, 32).transpose(1, 0, 2)  # [128, NSUB, 32]
        cnm = np.zeros((NCHUNK, 128, CHUNK * 32), np.float16)
        s0 = 0
        for cc, nsub in enumerate(SIZES):
            cnm[cc, :, :nsub * 32] = t[:, s0:s0 + nsub].reshape(128, nsub * 32)
            s0 += nsub
        in_maps.append({"xT": xt, "hc1": hc1, "cnm": cnm, "Wc": Wc, "Th": Th,
                        "wc2b": wc2b})
    return in_maps


def _decode(results):
    hs, cs = [], []
    for ci in range(N_CORES):
        out = np.asarray(results[ci]["hc_out"])  # [NCHUNK, 128, CHUNK*64] f16
        parts = []
        for cc, nsub in enumerate(SIZES):
            a = out[cc, :, :nsub * 64].reshape(128, nsub, 64)
            parts.append(a.transpose(1, 0, 2).reshape(nsub * 128, 64))
        arr = np.concatenate(parts, axis=0)[:NPC]
        hs.append(arr[:, 0:32].astype(np.float32))
        cs.append(arr[:, 32:64].astype(np.float32))
    return np.concatenate(hs, axis=0), np.concatenate(cs, axis=0)


def _run(inputs, trace=False):
    nc = build_nc()
    in_maps = _prep_inputs(
        inputs["x"], inputs["h"], inputs["c"], inputs["W_x"],
        inputs["Theta"], inputs["conv_b"], inputs["w_c"], inputs["b"])
    res = run_bass_kernel_spmd(nc, in_maps, list(range(N_CORES)), trace=trace)
    h_new, c_new = _decode(res.results)
    return (h_new, c_new), res


def kernel(**inputs):
    (h_new, c_new), _ = _run(inputs, trace=False)
    return (h_new, c_new)


def kernel_profiled(**inputs):
    (h_new, c_new), res = _run(inputs, trace=True)
    return (h_new, c_new), res
